# revision 1
# baseline (speedup 1.0000x reference)
"""CARAFE forward on 8 TRN2 NeuronCores.

Problem: features (8,128,64,64) f32, masks (8,25,128,128) f32
         -> out (8,128,128,128) f32, KERNEL_SIZE=5, GROUP=1, SCALE=2.

Sharding: pure data-parallel, one batch sample per core.

Formulation (banded matmul, i-pairs stacked along K):
  out[c, 2h+p, 2w+q] = sum_{i,j} f[c, h+i-2, w+j-2] * m[i*5+j, 2h+p, 2w+q]
For fixed (h, i) this is a matmul over x = w+j-2 (K=64):
  PSUM[c, col(p,w,q)] += sum_x f[c, r, x] * S(h,i)[x, col],  r = h+i-2
  S(h,i)[w+j-2, p*128+2w+q] = m[5i+j, 2h+p, 2w+q]  (banded; entries whose
  feature column is zero-padded are dropped).
Two consecutive i's share K=128 by stacking features of rows r and r+1
(partition x holds row r, partition 64+x holds row r+1). Per h: matmuls
for i-groups {0,1}, {2,3} (K=128, N=256 each) plus a K=64 matmul for i=4
whose band is packed even-h in partitions 0-63 / odd-h in 64-127 (the
stacked bottom half already holds row r+1, so the odd-h variant reads
partitions 64-127). All accumulate into a 256-column slice of an 8-h
PSUM mega-tile; PSUM is evacuated per 8 h's with a f32->fp16 cast copy
alternating between DVE and ACT, then one contiguous fp16 output DMA on
the ACT HWDGE ring (separate FIFO from the input loads); the host casts
the fp16 output back to f32.

Dataflow: inputs arrive as four merged 3.3 MB chunks — each carries the
feature rows (20-row halo window), the i-group bands, and the i=4 band
for its 16 output rows in one contiguous [128, 12800] fp16 transfer —
so DMAs run at the >=1 MiB line-rate knee AND each block's data lands
together, with everything SBUF-resident. All layout prep is host-side
numpy (no FLOPs).
"""

import numpy as np

N_CORES = 8
C, H, W = 128, 64, 64
K5 = 5
PAD = 2
KX = W                    # 64; K=128 after i-pair stacking
NCOL = 256                # (p, wo) output columns per low-res row h
HB = 8                    # h rows per PSUM mega-tile / evacuation block
FC = 16                   # h rows per merged input chunk
FTR = 20                  # lhsT rows per chunk (16 + 2 halo each side)
OFF_FT = 0
OFF_S2 = FTR * C                      # 2560
OFF_S3 = OFF_S2 + FC * 2 * NCOL       # 10752
MCW = OFF_S3 + (FC // 2) * NCOL       # 12800 elems/partition/chunk

_compiled = {}


def _emit_body(nc, mybir, ms, ps, ob, mc, out):
    """One full CARAFE sample: 4 merged input DMAs + 189 matmuls +
    8 PSUM evacuations + 8 output stores."""
    dt16 = mybir.dt.float16

    def h_mm01(h):
        mm = []
        if h == 1:
            mm.append((0, KX, 0))          # i=1 alone: lhsT top = f[:,0]
        elif h >= 2:
            mm.append((h - 2, 2 * KX, 0))  # i={0,1}: rows h-2, h-1
        mm.append((h, 2 * KX if h < H - 1 else KX, 1))  # i={2,3}
        return mm

    mc_ts = [ms.tile([2 * KX, MCW], dt16, tag="mc", name=f"mc_t{ci}")
             for ci in range(H // FC)]
    for ci in range(H // FC):
        nc.sync.dma_start(mc_ts[ci][:], mc[ci])

    for bi, b0 in enumerate(range(0, H, HB)):
        ci = b0 // FC
        h0 = FC * ci
        mt = mc_ts[ci]

        def lhs(ks, idx):
            t = idx - h0 + 2
            return mt[0:ks, OFF_FT + t * C:OFF_FT + (t + 1) * C]

        # Per-slice single-bank PSUM tiles (one accumulation group per
        # bank) with per-slice DVE/ACT-alternating evacuation into the
        # staged output tile: the PSUM drain pipelines behind the PE
        # instead of serializing per 8-h mega-tile.
        o = ob.tile([C, HB * NCOL], dt16, tag="o")
        for hl in range(HB):
            h = b0 + hl
            acc = ps.tile([C, NCOL], mybir.dt.float32)
            mms = h_mm01(h)
            n_mm = len(mms) + (1 if h + 2 < H else 0)
            for n_i, (hw, ks, g) in enumerate(mms):
                co = OFF_S2 + ((h - h0) * 2 + g) * NCOL
                nc.tensor.matmul(
                    acc[:], lhs(ks, hw), mt[0:ks, co:co + NCOL],
                    start=(n_i == 0), stop=(n_i == n_mm - 1))
            if h + 2 < H:
                # i=4: row h+2. even h -> top half, odd h -> bottom
                po = 0 if h % 2 == 0 else KX
                hw = h + 2 if h % 2 == 0 else h + 1
                t = hw - h0 + 2
                co = OFF_S3 + ((h - h0) // 2) * NCOL
                nc.tensor.matmul(
                    acc[:],
                    mt[po:po + KX, OFF_FT + t * C:OFF_FT + (t + 1) * C],
                    mt[po:po + KX, co:co + NCOL],
                    start=False, stop=True)
            o_sl = o[:, hl * NCOL:(hl + 1) * NCOL]
            if hl % 2 == 1:
                nc.scalar.copy(o_sl, acc[:])
            else:
                nc.vector.tensor_copy(o_sl, acc[:])
        # ACT HWDGE ring: outputs don't queue FIFO behind the input loads
        nc.scalar.dma_start(
            out[:, 2 * b0:2 * (b0 + HB), :],
            o[:].rearrange("c (hp w) -> c hp w", w=2 * W))


def _build(n_reps=1, loop_trip=None):
    """Unrolled program (n_reps bodies), or a For_i-looped one (timing)."""
    import concourse.bacc as bacc
    import concourse.mybir as mybir
    import concourse.tile as tile

    dt16 = mybir.dt.float16
    nc = bacc.Bacc("TRN2", target_bir_lowering=False, debug=False,
                   num_devices=N_CORES)
    mc = nc.dram_tensor("mc", [H // FC, 2 * KX, MCW], dt16,
                        kind="ExternalInput")
    out = nc.dram_tensor("out", [C, 2 * H, 2 * W], dt16,
                         kind="ExternalOutput")

    with tile.TileContext(nc) as tc:
        with (
            tc.tile_pool(name="ms", bufs=4) as ms,
            tc.tile_pool(name="ps", bufs=8, space="PSUM") as ps,
            tc.tile_pool(name="ob", bufs=4) as ob,
        ):
            def emit():
                _emit_body(nc, mybir, ms, ps, ob, mc, out)
            if loop_trip is not None:
                # PE body spans >1 IRAM block; hint the back-edge so the
                # timing loop doesn't pay an ifetch stall per iteration
                # (single-shot execution has no back-edge at all).
                with tc.For_i(0, loop_trip, 1,
                              hint_engines=(mybir.EngineType.PE,)):
                    emit()
            else:
                for _ in range(n_reps):
                    emit()
    nc.compile()
    return nc


def _band(masks_n, i):
    """S(h,i) banded matrix for all h: [KX, H, 2, W, 2] from one sample's
    masks [25, 2H, 2W]; S[w+j-2, h, p, w, q] = m[5i+j, 2h+p, 2w+q]."""
    m = masks_n.reshape(K5, K5, H, 2, W, 2)  # [i, j, h, p, w, q]
    s = np.zeros((KX, H, 2, W, 2), dtype=np.float16)
    for j in range(K5):
        wlo = max(0, PAD - j)
        whi = min(W, W + PAD - j)
        wi = np.arange(wlo, whi)
        s[wi + j - PAD, :, :, wi, :] = m[i, j, :, :, wlo:whi].transpose(
            2, 0, 1, 3)
    return s


def _prep_inputs(features: np.ndarray, masks: np.ndarray):
    """Host-side layout prep (no FLOPs): merged per-chunk tensor holding
    stacked feature rows, grouped banded S2, and h-interleaved i=4 band."""
    n = features.shape[0]
    ftw = features.transpose(0, 3, 2, 1).astype(np.float16)  # [n, w, h, c]
    ft2 = np.zeros((n, 2 * KX, H, C), dtype=np.float16)
    ft2[:, :KX] = ftw
    ft2[:, KX:, :H - 1] = ftw[:, :, 1:]      # row h+1; zero at h = H-1

    s2 = np.zeros((n, 2 * KX, H, 2, NCOL), dtype=np.float16)
    s3 = np.zeros((n, 2 * KX, H // 2, NCOL), dtype=np.float16)
    for smp in range(n):
        bands = [_band(masks[smp], i).reshape(KX, H, NCOL) for i in range(K5)]
        # group 0: i=0 (top, rows h-2 valid h>=2), i=1 (bottom, valid h>=1)
        s2[smp, :KX, 2:, 0] = bands[0][:, 2:]
        s2[smp, KX:, 2:, 0] = bands[1][:, 2:]
        s2[smp, :KX, 1, 0] = bands[1][:, 1]   # h=1 special: i=1 on top half
        # group 1: i=2 (top, always), i=3 (bottom, valid h <= H-2)
        s2[smp, :KX, :, 1] = bands[2]
        s2[smp, KX:, :H - 1, 1] = bands[3][:, :H - 1]
        # s3: i=4 (valid h <= H-3); even h -> partitions 0-63, odd -> 64-127
        b4 = bands[4]
        s3[smp, :KX, :, :] = b4[:, 0::2, :]
        s3[smp, KX:, :, :] = b4[:, 1::2, :]

    mc = np.empty((n, H // FC, 2 * KX, MCW), dtype=np.float16)
    for ci in range(H // FC):
        # feature rows FC*ci-2 .. FC*ci+17 (zero outside [0, H))
        fwin = np.zeros((n, 2 * KX, FTR, C), dtype=np.float16)
        lo = FC * ci - 2
        src_lo, src_hi = max(lo, 0), min(lo + FTR, H)
        fwin[:, :, src_lo - lo:src_hi - lo, :] = ft2[:, :, src_lo:src_hi, :]
        mc[:, ci, :, OFF_FT:OFF_S2] = fwin.reshape(n, 2 * KX, FTR * C)
        mc[:, ci, :, OFF_S2:OFF_S3] = s2[:, :, FC * ci:FC * (ci + 1)].reshape(
            n, 2 * KX, FC * 2 * NCOL)
        mc[:, ci, :, OFF_S3:] = s3[:, :, (FC // 2) * ci:(FC // 2) * (ci + 1)
                                   ].reshape(n, 2 * KX, (FC // 2) * NCOL)
    return mc


def kernel(features: np.ndarray, masks: np.ndarray) -> np.ndarray:
    from concourse.bass_utils import run_bass_kernel_spmd

    if 1 not in _compiled:
        _compiled[1] = _build(1)
    nc = _compiled[1]

    mc = _prep_inputs(np.asarray(features, dtype=np.float32),
                      np.asarray(masks, dtype=np.float32))
    in_maps = [{"mc": mc[i]} for i in range(N_CORES)]
    res = run_bass_kernel_spmd(nc, in_maps, list(range(N_CORES)))
    return np.stack([res.results[i]["out"].astype(np.float32)
                     for i in range(N_CORES)], axis=0)

